# revision 54
# baseline (speedup 1.0000x reference)
"""Trainium2 Bass kernel for the DSCNMP GNN (2x GINConv + pooling + MLP head).

Self-contained: takes full (unsharded) inputs, shards nodes/edges across the
8 NeuronCores internally, runs one SPMD Bass program via
bass_utils.run_bass_kernel_spmd, and returns the full [G, O] output.

Design (nodes partitioned contiguously; edges owned by their dst core):
  - conv1 aggregation: host-precomputed edge stream of positions (posE,
    bf16); per 128-edge slot a one-hot S (DVE is_equal) selects dst slots
    and the TensorEngine accumulates agg^T = posE^T @ S in PSUM.
  - x1 is AllGathered in four src-chunks (bf16 tables sized 512-multiples,
    <= 32768 rows for int16 gather indices); each chunk's AllGather is
    dispatched between the first gather calls so neither blocks the
    in-order GpSimd queue.
  - conv2 aggregation: per-edge x1 rows (256B) are fetched with
    gpsimd.dma_gather (SWDGE) from the chunk tables; edges are grouped
    into (src-chunk, dst-window-256) cells and the one-hot scatter matmul
    accumulates each window in PSUM.  The sweep is chunk-major so chunk c
    only needs AllGather c by call 5c; the last chunk's pass finalizes
    windows (conv2 MLP, transpose, graph pool) as it goes.
  - The scheduler's SWDGE cost constant is raised to the measured HW rate
    during build so the static schedule reflects real gather time, and
    sweep consumers carry tile_wait_until stamps at their gather's real
    completion so conv1 never head-of-line blocks behind a landing wait.
  - Graph pooling uses host-precomputed one-hot B (Bpool); the pooled
    [*, GWIN] per-core windows are combined with small AllGathers plus
    locally-baked shifted adds (all 8 graph offsets are host-known), not
    AllReduces, which contend with the gather DMA stream.
  - The graph-level MLPs run mid-sweep as soon as their collective lands;
    only the x2-dependent tail remains after the last gather.
"""

import numpy as np

N_FULL, E_FULL, G_FULL, C_DIM, H_DIM, O_DIM = 100000, 600000, 1000, 2, 128, 10
HC_DIM = H_DIM // 2
NCORES = 8
NCHUNK = 4          # int16 gather-index chunking of the global table
WWIN = 256          # conv2 dst-window width
WPG = 10            # windows per gather call
EPS = 1e-5

_CACHE = {}


def _pack_idx16(flat):
    """[j%16, j//16] int16 packing, replicated across the 8 Q7 groups."""
    total = len(flat)
    assert total % 16 == 0
    out = flat.reshape(total // 16, 16).T.astype(np.int16)
    return np.tile(out, (8, 1))


def _preprocess(pos, edge_index, batch, N, E, G):
    NL = N // NCORES
    NLP = -(-NL // 512) * 512
    NT = NLP // 128
    # asymmetric src-chunks: a small chunk 0 lets AllGather 0 (and thus the
    # first conv2 gather) start much earlier; chunks stay under the int16
    # gather-index limit (rows*NCORES <= 32768)
    QB = (0, 2560, 2560 + 3584, 2560 + 2 * 3584, NLP)
    QLS = tuple(QB[i + 1] - QB[i] for i in range(NCHUNK))
    NWIN = NLP // WWIN
    assert NWIN % WPG == 0
    assert all(q * NCORES <= 32767 + 1 for q in QLS)
    assert all(q % 512 == 0 for q in QLS)

    pos = np.asarray(pos, np.float32)
    src = np.asarray(edge_index[0], np.int64)
    dst = np.asarray(edge_index[1], np.int64)
    batch = np.asarray(batch, np.int64)
    assert N % NCORES == 0

    node = np.arange(N)
    slot_of = NLP * (node // NL) + (node % NL)

    pos_nm = np.zeros((NCORES, 128, NT * C_DIM), np.float32)
    batch_rel = np.full((NCORES, 128, NT), -5.0, np.float32)
    g0 = np.zeros(NCORES, np.int64)
    gwin_need = 0
    for k in range(NCORES):
        nodes = np.arange(k * NL, (k + 1) * NL)
        j = nodes - k * NL
        pos_nm[k][j % 128, (j // 128) * C_DIM + 0] = pos[nodes, 0]
        pos_nm[k][j % 128, (j // 128) * C_DIM + 1] = pos[nodes, 1]
        g0[k] = batch[nodes[0]]
        rel = batch[nodes] - g0[k]
        batch_rel[k][j % 128, j // 128] = rel.astype(np.float32)
        gwin_need = max(gwin_need, int(rel.max()) + 1)
    GWIN = min(512, max(128, -(-gwin_need // 32) * 32))
    assert gwin_need <= GWIN <= 512
    WG = -(-(G + GWIN) // 256) * 256

    ecore = dst // NL
    ksrc = slot_of[src] // NLP
    jsrc = slot_of[src] % NLP
    qsrc = np.searchsorted(np.array(QB), jsrc, side="right") - 1
    qls_arr = np.array(QLS)[qsrc]
    qb_arr = np.array(QB)[qsrc]
    grow_q = ksrc * qls_arr + (jsrc - qb_arr)   # row within chunk-table
    dslot = slot_of[dst] % NLP

    # ---- conv1 edge stream: cells = dst tile only, t-major ----
    tt_all = dslot // 128
    cells1 = [[None] * NT for _ in range(NCORES)]
    cnt1 = np.zeros((NCORES, NT), np.int64)
    for k in range(NCORES):
        m = ecore == k
        sk, dk = src[m], dslot[m]
        tk = dk // 128
        for t in range(NT):
            mm = tk == t
            dd = dk[mm]
            o = np.argsort(dd, kind="stable")
            cells1[k][t] = (sk[mm][o], dd[o] - t * 128)
            cnt1[k, t] = len(dd)
    slots1 = [-(-int(cnt1[:, t].max()) // 128) for t in range(NT)]
    off1 = np.concatenate([[0], np.cumsum(slots1)]).astype(np.int64)
    NSLOT1 = int(off1[-1])

    posE = np.zeros((NCORES, 128, NSLOT1 * C_DIM), np.float32)
    dwc1 = np.full((NCORES, 128, NSLOT1), -5.0, np.float32)
    for k in range(NCORES):
        pe = np.zeros((NSLOT1 * 128, C_DIM), np.float32)
        dw = np.full(NSLOT1 * 128, -5.0, np.float32)
        for t in range(NT):
            ss, dd = cells1[k][t]
            base = int(off1[t]) * 128
            pe[base:base + len(ss)] = pos[ss]
            dw[base:base + len(dd)] = dd.astype(np.float32)
        posE[k] = pe.reshape(NSLOT1, 128, C_DIM).transpose(1, 0, 2).reshape(
            128, NSLOT1 * C_DIM)
        dwc1[k] = dw.reshape(NSLOT1, 128).T

    # ---- conv2 edge stream: cells = (src quarter, dst window-512) ----
    ww_all = dslot // WWIN
    cells2 = [[[None] * NWIN for _ in range(NCHUNK)] for _ in range(NCORES)]
    cnt2 = np.zeros((NCORES, NCHUNK, NWIN), np.int64)
    for k in range(NCORES):
        m = ecore == k
        gq, ds_, ch, wk = grow_q[m], dslot[m], qsrc[m], ww_all[m]
        for c in range(NCHUNK):
            for w in range(NWIN):
                mm = (ch == c) & (wk == w)
                gg, dd = gq[mm], ds_[mm]
                o = np.argsort(dd, kind="stable")
                cells2[k][c][w] = (gg[o], dd[o] - w * WWIN)
                cnt2[k, c, w] = len(gg)
    slots2 = [[-(-int(cnt2[:, c, w].max()) // 128) for w in range(NWIN)]
              for c in range(NCHUNK)]
    off2 = np.concatenate(
        [[0], np.cumsum(np.array(slots2).reshape(-1))]).astype(np.int64)
    NSLOT2 = int(off2[-1])

    gidx2 = np.zeros((NCORES, 128, NSLOT2 * 8), np.int16)
    dwc2 = np.full((NCORES, 128, NSLOT2), -5, np.int16)
    for k in range(NCORES):
        gi = np.zeros(NSLOT2 * 128, np.int64)
        dw = np.full(NSLOT2 * 128, -5, np.int16)
        for c in range(NCHUNK):
            for w in range(NWIN):
                gg, dd = cells2[k][c][w]
                base = int(off2[c * NWIN + w]) * 128
                gi[base:base + len(gg)] = gg
                dw[base:base + len(dd)] = dd.astype(np.int16)
        gidx2[k] = _pack_idx16(gi)
        dwc2[k] = dw.reshape(NSLOT2, 128).T

    # host-precomputed one-hot matrices (all static given the graph):
    #   Bpool[k][p, t*GWIN+g] = 1 if batch_rel[k][p,t] == g   (graph pooling)
    #   S8h[k][p, s*128+j]    = 1 if dwc1[k][p,s] == j        (conv1 scatter)
    #   S2h[k][p, s*WWIN+j]   = 1 if dwc2[k][p,s] == j        (conv2 scatter)
    import ml_dtypes
    bf = ml_dtypes.bfloat16
    Bpool = (batch_rel[:, :, :, None] ==
             np.arange(GWIN, dtype=np.float32)).astype(bf).reshape(
        NCORES, 128, NT * GWIN)

    groff = np.zeros((NCORES, 1, 2), np.int32)
    groff[:, 0, 0] = g0
    assert (g0 + GWIN <= WG).all()

    dims = dict(N=N, E=E, G=G, NL=NL, NLP=NLP, NT=NT, QLS=QLS, NWIN=NWIN,
                NSLOT1=NSLOT1, NSLOT2=NSLOT2, GWIN=GWIN, WG=WG,
                g0all=tuple(int(x) for x in g0),
                slots1=tuple(slots1),
                slots2=tuple(tuple(r) for r in slots2))
    arrays = dict(posE=posE, pos_nm=pos_nm, batch_rel=batch_rel,
                  dwc1=dwc1, gidx2=gidx2, dwc2=dwc2, groff=groff,
                  Bpool=Bpool)
    return dims, arrays


def _build_program(dims):
    from concourse import hw_specs
    old_rate = hw_specs.TRN2Spec.SWDGE_NS_PER_DESCRIPTOR
    hw_specs.TRN2Spec.SWDGE_NS_PER_DESCRIPTOR = 7.5
    try:
        return _build_program_inner(dims)
    finally:
        hw_specs.TRN2Spec.SWDGE_NS_PER_DESCRIPTOR = old_rate


def _build_program_inner(dims):
    import contextlib
    import concourse.bass as bass
    import concourse.bacc as bacc
    import concourse.mybir as mybir
    import concourse.tile as tile
    from concourse import library_config
    from concourse.masks import make_identity

    f32 = mybir.dt.float32
    bf16 = mybir.dt.bfloat16
    i16 = mybir.dt.int16
    i32 = mybir.dt.int32
    AF = mybir.ActivationFunctionType
    ALU = mybir.AluOpType

    NLP, NT, NWIN = dims["NLP"], dims["NT"], dims["NWIN"]
    QLS = list(dims["QLS"])
    QB = [0]
    for q in QLS:
        QB.append(QB[-1] + q)
    G0ALL = list(dims["g0all"])
    NSLOT1, NSLOT2 = dims["NSLOT1"], dims["NSLOT2"]
    GWIN, WG, G = dims["GWIN"], dims["WG"], dims["G"]
    slots1 = list(dims["slots1"])
    slots2 = [list(r) for r in dims["slots2"]]
    off1 = np.concatenate([[0], np.cumsum(slots1)]).astype(np.int64)
    off2 = np.concatenate(
        [[0], np.cumsum(np.array(slots2).reshape(-1))]).astype(np.int64)
    QTB = [b // 128 for b in QB]        # tile bounds per quarter
    WQB = [b // 512 for b in QB]        # 512-window bounds per quarter
    NWG = NWIN // WPG                   # gather groups per chunk
    MAXSL1 = max(slots1)
    MAXCALL = max(sum(slots2[c][wg * WPG:(wg + 1) * WPG])
                  for c in range(NCHUNK) for wg in range(NWG))
    MAXSL2 = max(max(r) for r in slots2)
    # uniform 512-wide conv1 MLP windows (quarters are 512-multiples)
    wins = [(i * 512, 512) for i in range(NLP // 512)]

    nc = bacc.Bacc("TRN2", target_bir_lowering=False, debug=False,
                   enable_asserts=True, num_devices=NCORES)

    def din(name, shape, dt=f32):
        return nc.dram_tensor(name, list(shape), dt, kind="ExternalInput")

    posE_d = din("posE", [128, NSLOT1 * C_DIM], bf16)
    pos_nm_d = din("pos_nm", [128, NT * C_DIM], bf16)
    dwc1_d = din("dwc1", [128, NSLOT1], bf16)
    gidx2_d = din("gidx2", [128, NSLOT2 * 8], i16)
    dwc2_d = din("dwc2", [128, NSLOT2], i16)
    iota_d = din("iota", [128, 128], bf16)
    iota16_d = din("iota16", [128, WWIN], i16)
    Bpool_d = din("Bpool", [128, NT * GWIN], bf16)

    wnames = {}
    for nm, shp in [("W1a", [C_DIM, H_DIM]), ("W1b", [H_DIM, H_DIM]),
                    ("W2a", [H_DIM, H_DIM]), ("W2b", [H_DIM, H_DIM]),
                    ("Wf1", [C_DIM, H_DIM]), ("Wf2", [H_DIM, H_DIM]),
                    ("Wc1", [H_DIM, HC_DIM]), ("Wc2", [HC_DIM, O_DIM])]:
        wnames[nm] = din(nm, shp)
    vecs = {}
    for nm in ["b1a", "b1b", "b2a", "b2b", "bf1", "bf2",
               "n1_g", "n1_b", "n1_rm", "n1_rv", "n2_g", "n2_b", "n2_rm", "n2_rv",
               "f1_g", "f1_b", "f1_rm", "f1_rv", "f2_g", "f2_b", "f2_rm", "f2_rv"]:
        vecs[nm] = din(nm, [H_DIM, 1])
    for nm in ["bc1", "gc", "bec", "rmc", "rvc", "a_prelu_v"]:
        vecs[nm] = din(nm, [HC_DIM, 1])
    vecs["bc2"] = din("bc2", [O_DIM, 1])

    out_d = nc.dram_tensor("out", [G, O_DIM], f32, kind="ExternalOutput")

    with tile.TileContext(nc) as tc:
        nc.gpsimd.load_library(library_config.mlp)
        ctx = contextlib.ExitStack()
        with ctx:
            dram = ctx.enter_context(tc.tile_pool(name="dram", bufs=1, space="DRAM"))
            pconst = ctx.enter_context(tc.tile_pool(name="const", bufs=1))
            pbig = ctx.enter_context(tc.tile_pool(name="big", bufs=1))
            pland = ctx.enter_context(tc.tile_pool(name="land", bufs=6))
            psmall = ctx.enter_context(tc.tile_pool(name="small", bufs=2))
            ps1 = ctx.enter_context(tc.tile_pool(name="s1", bufs=2))
            pz = ctx.enter_context(tc.tile_pool(name="z", bufs=2))
            pgr = ctx.enter_context(tc.tile_pool(name="gr", bufs=1))
            ph1 = ctx.enter_context(tc.tile_pool(name="h1w", bufs=2))
            ppsum = ctx.enter_context(tc.tile_pool(name="psum", bufs=2, space="PSUM"))
            pseg = ctx.enter_context(tc.tile_pool(name="psum_seg", bufs=2, space="PSUM"))
            ppool = ctx.enter_context(tc.tile_pool(name="psum_acc", bufs=2, space="PSUM"))

            cc_in = [dram.tile([QLS[q], H_DIM], bf16, tag="cc_in",
                                name=f"cc_in{q}", bufs=NCHUNK)
                     for q in range(NCHUNK)]
            cc_out = [dram.tile([QLS[q] * NCORES, H_DIM], bf16, tag="cc_out",
                                name=f"cc_out{q}", addr_space="Shared",
                                bufs=NCHUNK) for q in range(NCHUNK)]
            ar1_in = dram.tile([H_DIM + C_DIM, GWIN], f32, tag="ar1_in")
            ar1_out = dram.tile([(H_DIM + C_DIM) * NCORES, GWIN], f32,
                                tag="ar1_out", addr_space="Shared")
            ar2_in = dram.tile([H_DIM, GWIN], f32, tag="ar2_in")
            ar2_out = dram.tile([H_DIM * NCORES, GWIN], f32, tag="ar2_out",
                                addr_space="Shared")

            def load_const(dr, shape, dt=f32):
                t = pconst.tile(shape, dt, tag=dr.name + "_sb")
                nc.sync.dma_start(out=t[:], in_=dr.ap())
                return t

            W = {k: load_const(v, v.shape) for k, v in wnames.items()}
            V = {k: load_const(v, v.shape) for k, v in vecs.items()}
            pos_nm = load_const(pos_nm_d, [128, NT * C_DIM], bf16)
            posE = load_const(posE_d, [128, NSLOT1 * C_DIM], bf16)
            gidx2 = load_const(gidx2_d, [128, NSLOT2 * 8], i16)
            dwc1 = load_const(dwc1_d, [128, NSLOT1], bf16)
            dwc2 = load_const(dwc2_d, [128, NSLOT2], i16)
            iota_bf = load_const(iota_d, [128, 128], bf16)
            iota16 = load_const(iota16_d, [128, WWIN], i16)
            Bpool = load_const(Bpool_d, [128, NT * GWIN], bf16)

            ident = pconst.tile([128, 128], f32, tag="ident")
            make_identity(nc, ident[:])
            ident_bf = pconst.tile([128, 128], bf16, tag="ident_bf")
            nc.vector.tensor_copy(ident_bf[:], ident[:])

            def bn_vec(g, b, rm, rv, P, nm):
                a = pconst.tile([P, 1], f32, tag=f"bn_a_{nm}")
                c = pconst.tile([P, 1], f32, tag=f"bn_c_{nm}")
                nc.vector.tensor_scalar(a[:], rv[:], EPS, None, ALU.add)
                nc.scalar.activation(a[:], a[:], AF.Sqrt)
                nc.vector.reciprocal(a[:], a[:])
                nc.vector.tensor_tensor(a[:], a[:], g[:], op=ALU.mult)
                nc.vector.tensor_tensor(c[:], rm[:], a[:], op=ALU.mult)
                nc.vector.tensor_tensor(c[:], b[:], c[:], op=ALU.subtract)
                return a, c
            a1, c1 = bn_vec(V["n1_g"], V["n1_b"], V["n1_rm"], V["n1_rv"], H_DIM, "n1")
            a2, c2 = bn_vec(V["n2_g"], V["n2_b"], V["n2_rm"], V["n2_rv"], H_DIM, "n2")
            af1, cf1 = bn_vec(V["f1_g"], V["f1_b"], V["f1_rm"], V["f1_rv"], H_DIM, "f1")
            af2, cf2 = bn_vec(V["f2_g"], V["f2_b"], V["f2_rm"], V["f2_rv"], H_DIM, "f2")
            acl, ccl = bn_vec(V["gc"], V["bec"], V["rmc"], V["rvc"], HC_DIM, "cls")

            W1a_bf = pconst.tile([C_DIM, H_DIM], bf16, tag="W1a_bf")
            nc.vector.tensor_copy(W1a_bf[:], W["W1a"][:])
            W1b_bf = pconst.tile([H_DIM, H_DIM], bf16, tag="W1b_bf")
            nc.vector.tensor_copy(W1b_bf[:], W["W1b"][:])
            Wf1_bf = pconst.tile([C_DIM, H_DIM], bf16, tag="Wf1_bf")
            nc.vector.tensor_copy(Wf1_bf[:], W["Wf1"][:])
            Wf2_bf = pconst.tile([H_DIM, H_DIM], bf16, tag="Wf2_bf")
            nc.vector.tensor_copy(Wf2_bf[:], W["Wf2"][:])
            Wc1_bf = pconst.tile([H_DIM, HC_DIM], bf16, tag="Wc1_bf")
            nc.vector.tensor_copy(Wc1_bf[:], W["Wc1"][:])
            Wc2_bf = pconst.tile([HC_DIM, O_DIM], bf16, tag="Wc2_bf")
            nc.vector.tensor_copy(Wc2_bf[:], W["Wc2"][:])
            W2a_bf = pconst.tile([H_DIM, H_DIM], bf16, tag="W2a_bf")
            nc.vector.tensor_copy(W2a_bf[:], W["W2a"][:])
            W2b_bf = pconst.tile([H_DIM, H_DIM], bf16, tag="W2b_bf")
            nc.vector.tensor_copy(W2b_bf[:], W["W2b"][:])

            # persistent big buffers
            xT = pbig.tile([128, NLP], bf16, tag="B")        # x1T -> h2T -> x2T
            xnm = pbig.tile([128, NT * H_DIM], bf16, tag="NM")  # x1nm then x2nm

            posE_v = posE[:].rearrange("p (s c) -> p s c", c=C_DIM)
            h1w = {}

            # =============== phase 1: conv1, per quarter ===============
            for q in range(4):
                for t in range(QTB[q], QTB[q + 1]):
                    ns = slots1[t]
                    s0 = int(off1[t])
                    ps = pseg.tile([C_DIM, 128], f32, tag="seg1", bufs=1)
                    if ns:
                        S8 = ps1.tile([128, MAXSL1 * 128], bf16, tag="S8")
                        nc.vector.tensor_tensor(
                            out=S8[:, 0:ns * 128].rearrange(
                                "p (s j) -> p s j", j=128),
                            in0=iota_bf[:, None, 0:128].to_broadcast(
                                [128, ns, 128]),
                            in1=dwc1[:, s0:s0 + ns, None].to_broadcast(
                                [128, ns, 128]),
                            op=ALU.is_equal)
                        for sl in range(ns):
                            nc.tensor.matmul(ps[:], posE_v[:, s0 + sl, :],
                                             S8[:, sl * 128:(sl + 1) * 128],
                                             start=(sl == 0), stop=False)
                    nc.tensor.matmul(ps[:], pos_nm[:, t * C_DIM:(t + 1) * C_DIM],
                                     ident_bf[:], start=(ns == 0), stop=True)
                    # flush into the h1 window (bf16)
                    wi = t // 4
                    o = (t % 4) * 128
                    if wi not in h1w:
                        h1w[wi] = ph1.tile([C_DIM, wins[wi][1]], bf16,
                                           tag=f"h1w{wins[wi][1]}", name=f"h1w{wi}")
                    nc.scalar.copy(h1w[wi][:, o:o + 128], ps[0:C_DIM, :])
                # conv1 MLP for this quarter's windows
                for wi in range(WQB[q], WQB[q + 1]):
                    c0, cw = wins[wi]
                    psm = ppsum.tile([H_DIM, 512], f32, tag="work")
                    nc.tensor.matmul(psm[:, 0:cw], W1a_bf[:], h1w[wi][:],
                                     start=True, stop=True)
                    zt = pz.tile([128, 512], bf16, tag="z")
                    nc.scalar.activation(zt[:, 0:cw], psm[:, 0:cw],
                                         AF.Relu, bias=V["b1a"][:], scale=1.0)
                    psm2 = ppsum.tile([H_DIM, 512], f32, tag="work")
                    nc.tensor.matmul(psm2[:, 0:cw], W1b_bf[:], zt[:, 0:cw],
                                     start=True, stop=True)
                    nc.scalar.activation(psm2[:, 0:cw], psm2[:, 0:cw], AF.Relu,
                                         bias=V["b1b"][:], scale=1.0)
                    nc.vector.tensor_scalar(xT[:, c0:c0 + cw], psm2[:, 0:cw],
                                            a1[:], c1[:], ALU.mult, ALU.add)
                # transpose to node-major + AllGather this quarter
                for t in range(QTB[q], QTB[q + 1]):
                    pt = ppsum.tile([128, 128], bf16, tag="work")
                    nc.tensor.transpose(pt[:], xT[:, t * 128:(t + 1) * 128],
                                        ident_bf[:])
                    nc.scalar.copy(xnm[:, t * 128:(t + 1) * 128], pt[:])
                nqt = QTB[q + 1] - QTB[q]
                th = nqt // 2
                for h0, h1 in ((0, th), (th, nqt)):
                    nc.sync.dma_start(
                        out=cc_in[q][h0 * 128:h1 * 128, :].rearrange(
                            "(s p) f -> p s f", p=128),
                        in_=xnm[:, (QTB[q] + h0) * H_DIM:(QTB[q] + h1) * H_DIM
                                ].rearrange("p (s f) -> p s f", f=H_DIM))

            def ag_dispatch(q):
                nc.gpsimd.collective_compute(
                    "AllGather", mybir.AluOpType.bypass,
                    ins=[cc_in[q].opt()], outs=[cc_out[q].opt()],
                    replica_groups=[list(range(NCORES))])

            # =============== pools of pos and x1 (interleaved into sweep) ====
            ps_pos = ppool.tile([C_DIM, GWIN], f32, tag="accp", bufs=1)
            ps_x1 = ppool.tile([128, GWIN], f32, tag="acc")
            arin_pos = pgr.tile([C_DIM, GWIN], f32, tag="arin_p")
            arin_x1 = pgr.tile([H_DIM, GWIN], f32, tag="arin", bufs=2)
            pool_t = [0]

            def get_B(t, who):
                return Bpool[:, t * GWIN:(t + 1) * GWIN]

            def pool_step():
                t = pool_t[0]
                if t >= NT:
                    return
                Bt = get_B(t, "pool")
                nc.tensor.matmul(ps_pos[:], pos_nm[:, t * C_DIM:(t + 1) * C_DIM],
                                 Bt, start=(t == 0), stop=(t == NT - 1))
                nc.tensor.matmul(ps_x1[:], xnm[:, t * 128:(t + 1) * 128], Bt,
                                 start=(t == 0), stop=(t == NT - 1))
                pool_t[0] += 1
                if pool_t[0] == NT:
                    nc.scalar.copy(arin_pos[:], ps_pos[:])
                    nc.scalar.copy(arin_x1[:], ps_x1[:])

            # =============== conv2: gather + window seg-sum ===============
            ps_x2 = ppool.tile([128, GWIN], f32, tag="acc")
            tglob = [0]

            def finalize_block(wb):
                # h2 columns [512*wb, 512*(wb+1)) complete -> conv2 MLP ->
                # x2 -> pool
                c0 = wb * 512
                psm = ppsum.tile([H_DIM, 512], f32, tag="work")
                nc.tensor.matmul(psm[:], W2a_bf[:], xT[:, c0:c0 + 512],
                                 start=True, stop=True)
                zt = pz.tile([128, 512], bf16, tag="z")
                nc.scalar.activation(zt[:], psm[:],
                                     AF.Relu, bias=V["b2a"][:], scale=1.0)
                psm2 = ppsum.tile([H_DIM, 512], f32, tag="work")
                nc.tensor.matmul(psm2[:], W2b_bf[:], zt[:],
                                 start=True, stop=True)
                nc.scalar.activation(psm2[:], psm2[:], AF.Relu,
                                     bias=V["b2b"][:], scale=1.0)
                nc.vector.tensor_scalar(xT[:, c0:c0 + 512], psm2[:],
                                        a2[:], c2[:], ALU.mult, ALU.add)
                for t in range(wb * 4, wb * 4 + 4):
                    tg = tglob[0]
                    pt = ppsum.tile([128, 128], bf16, tag="work")
                    nc.tensor.transpose(pt[:], xT[:, t * 128:(t + 1) * 128],
                                        ident_bf[:])
                    nc.scalar.copy(xnm[:, t * 128:(t + 1) * 128], pt[:])
                    nc.tensor.matmul(ps_x2[:], xnm[:, t * 128:(t + 1) * 128],
                                     get_B(t, "fin"),
                                     start=(tg == 0), stop=(tg == NT - 1))
                    tglob[0] += 1

            def g_mlp(lhsT_w, rhs, out, bias, bn, P=H_DIM, relu=True):
                for w in range(-(-WG // 512)):
                    c0 = w * 512
                    cw = min(512, WG - c0)
                    ps = ppsum.tile([P, 512], f32, tag="work")
                    nc.tensor.matmul(ps[:P, :cw], lhsT_w[:], rhs[:, c0:c0 + cw],
                                     start=True, stop=True)
                    fn = AF.Relu if relu else AF.Identity
                    nc.scalar.activation(ps[:P, :cw], ps[:P, :cw], fn,
                                         bias=bias[:], scale=1.0)
                    if bn is not None:
                        a_, c_ = bn
                        nc.vector.tensor_scalar(out[:, c0:c0 + cw], ps[:P, :cw],
                                                a_[:], c_[:], ALU.mult, ALU.add)
                    else:
                        nc.scalar.copy(out[:, c0:c0 + cw], ps[:P, :cw])

            ar1x = pgr.tile([H_DIM, WG], bf16, tag="arbig", bufs=2)
            ar1p = pgr.tile([C_DIM, WG], bf16, tag="ar1p")
            x0g = pgr.tile([H_DIM, WG], bf16, tag="g_x0g")
            tmp = pgr.tile([H_DIM, WG], bf16, tag="g_tmp")
            x1g = pgr.tile([H_DIM, WG], bf16, tag="g_x1g", bufs=2)
            plw = ctx.enter_context(tc.tile_pool(name="lw", bufs=6))

            def ar1_seq():
                # pooled pos/x1: AllGather the per-core [H+C, GWIN] windows
                # (cheap, and unlike AllReduce it does not block the SWDGE
                # gather stream), then sum the 8 windows locally at the
                # compile-time-known graph offsets.
                nc.sync.dma_start(out=ar1_in[0:H_DIM, :], in_=arin_x1[:])
                nc.sync.dma_start(out=ar1_in[H_DIM:, :], in_=arin_pos[:])
                nc.gpsimd.collective_compute(
                    "AllGather", mybir.AluOpType.bypass,
                    ins=[ar1_in.opt()], outs=[ar1_out.opt()],
                    replica_groups=[list(range(NCORES))])

            def ar1_unpack():
                nc.vector.memset(ar1x[:], 0.0)
                nc.vector.memset(ar1p[:], 0.0)
                for k in range(NCORES):
                    r0 = k * (H_DIM + C_DIM)
                    lw = plw.tile([H_DIM, GWIN], f32, tag="lw")
                    nc.sync.dma_start(out=lw[:],
                                      in_=ar1_out[r0:r0 + H_DIM, :])
                    lp = plw.tile([C_DIM, GWIN], f32, tag="lp")
                    nc.sync.dma_start(out=lp[:],
                                      in_=ar1_out[r0 + H_DIM:r0 + H_DIM + C_DIM, :])
                    g0k = G0ALL[k]
                    nc.vector.tensor_tensor(ar1x[:, g0k:g0k + GWIN],
                                            ar1x[:, g0k:g0k + GWIN], lw[:],
                                            op=ALU.add)
                    nc.vector.tensor_tensor(ar1p[:, g0k:g0k + GWIN],
                                            ar1p[:, g0k:g0k + GWIN], lp[:],
                                            op=ALU.add)

            def graph_early():
                # x0g / x1g only need AllReduce 1 -> compute them during the
                # sweep so the post-AR2 tail is just one MLP + the cls head.
                g_mlp(Wf1_bf, ar1p[:], x0g, V["bf1"], (af1, cf1))
                nc.vector.tensor_tensor(tmp[:], x0g[:], ar1x[:], op=ALU.add)
                g_mlp(Wf2_bf, tmp, x1g, V["bf2"], (af2, cf2))
                nc.vector.tensor_tensor(tmp[:], x0g[:], x1g[:], op=ALU.add)

            # chunk 0 first (hides AllGathers 1-3), then round-robin chunks
            # 1-3 per window group so the finalize work (at chunk 3) spreads
            # across the sweep instead of piling into the tail
            sweep = [(c, wg * WPG, WPG) for c in range(NCHUNK)
                      for wg in range(NWG)]
            # AllGather dispatches are interleaved between gather calls so
            # neither ever head-of-line-blocks the other on the in-order
            # GpSimd queue: AG0 before gather 0, then AG1/AG2/AG3 spread
            # through the chunk-0 sweep (their inputs are ready by then).
            # The AR1 sequence rides the same queue once the pools are done,
            # and the AR1-dependent graph MLPs are emitted two groups later.
            ag_points = {0: [0], 1: [1], 2: [2], 3: [3]}
            for g_n, (c, w0, nw) in enumerate(sweep):
                    for q_ in ag_points.get(g_n, []):
                        ag_dispatch(q_)
                    if g_n == 6:
                        ar1_seq()
                    if g_n == 12:
                        ar1_unpack()
                        graph_early()
                    sbase = int(off2[c * NWIN + w0])
                    nsl_call = sum(slots2[c][w0:w0 + nw])
                    land = pland.tile([128, MAXCALL, H_DIM], bf16, tag="land")
                    nc.gpsimd.dma_gather(
                        land[:, 0:nsl_call, :], cc_out[c][:],
                        gidx2[:, sbase * 8:(sbase + nsl_call) * 8],
                        nsl_call * 128, nsl_call * 128, H_DIM,
                        single_packet=False)
                    loff = 0
                    # tell the scheduler when this land tile really arrives
                    # (HW dma_gather is ~38us/call); consumers scheduled at
                    # that point interleave cleanly with conv1 instead of
                    # head-of-line blocking the PE queue.
                    cons_ctx = tc.tile_wait_until(0.145 + 0.038 * (g_n + 1))
                    cons_ctx.__enter__()
                    for w in range(w0, w0 + nw):
                        if c == 0:
                            pool_step()
                            pool_step()
                        ns = slots2[c][w]
                        s0 = int(off2[c * NWIN + w])
                        if ns == 0:
                            if c == NCHUNK - 1 and w % 2 == 1:
                                finalize_block(w // 2)
                            continue
                        S = psmall.tile([128, MAXSL2 * WWIN], bf16, tag="S2")
                        nc.vector.tensor_tensor(
                            out=S[:, 0:ns * WWIN].rearrange(
                                "p (s j) -> p s j", j=WWIN),
                            in0=iota16[:, None, 0:WWIN].to_broadcast(
                                [128, ns, WWIN]),
                            in1=dwc2[:, s0:s0 + ns, None].to_broadcast(
                                [128, ns, WWIN]),
                            op=ALU.is_equal)
                        ps = pseg.tile([128, WWIN], f32, tag="seg2")
                        for sl in range(ns):
                            nc.tensor.matmul(ps[:], land[:, loff + sl, :],
                                             S[:, sl * WWIN:(sl + 1) * WWIN],
                                             start=(sl == 0), stop=(sl == ns - 1))
                        loff += ns
                        cols = slice(w * WWIN, (w + 1) * WWIN)
                        nc.vector.tensor_tensor(xT[:, cols], xT[:, cols], ps[:],
                                                op=ALU.add)
                        if c == NCHUNK - 1 and w % 2 == 1:
                            finalize_block(w // 2)
                    cons_ctx.__exit__(None, None, None)

            # x2 pool -> AllGather 2 + local shifted adds
            arin_x2 = pgr.tile([H_DIM, GWIN], f32, tag="arin", bufs=2)
            nc.scalar.copy(arin_x2[:], ps_x2[:])
            nc.sync.dma_start(out=ar2_in[:], in_=arin_x2[:])
            nc.gpsimd.collective_compute(
                "AllGather", mybir.AluOpType.bypass,
                ins=[ar2_in.opt()], outs=[ar2_out.opt()],
                replica_groups=[list(range(NCORES))])

            # =============== graph stage (post-AG2 tail) ===============
            ar2x = pgr.tile([H_DIM, WG], bf16, tag="arbig", bufs=2)
            nc.vector.memset(ar2x[:], 0.0)
            for k in range(NCORES):
                lw2 = plw.tile([H_DIM, GWIN], f32, tag="lw")
                nc.sync.dma_start(out=lw2[:],
                                  in_=ar2_out[k * H_DIM:(k + 1) * H_DIM, :])
                g0k = G0ALL[k]
                nc.vector.tensor_tensor(ar2x[:, g0k:g0k + GWIN],
                                        ar2x[:, g0k:g0k + GWIN], lw2[:],
                                        op=ALU.add)
            nc.vector.tensor_tensor(tmp[:], tmp[:], ar2x[:], op=ALU.add)
            x2g = pgr.tile([H_DIM, WG], bf16, tag="g_x0g")
            g_mlp(Wf2_bf, tmp, x2g, V["bf2"], (af2, cf2))

            hcls = pgr.tile([HC_DIM, WG], bf16, tag="g_tmp")
            g_mlp(Wc1_bf, x2g, hcls, V["bc1"], (acl, ccl), P=HC_DIM, relu=False)
            hneg = pgr.tile([HC_DIM, WG], bf16, tag="g_x1g", bufs=2)
            nc.vector.tensor_scalar(hneg[:], hcls[:], V["a_prelu_v"][:], None,
                                    ALU.mult)
            nc.vector.tensor_tensor(hcls[:], hcls[:], hneg[:], op=ALU.max)
            outT = pgr.tile([O_DIM, WG], bf16, tag="g_x1g", bufs=2)
            g_mlp(Wc2_bf, hcls, outT, V["bc2"], None, P=O_DIM, relu=False)

            ngt = -(-G // 128)
            onm = pgr.tile([128, ngt * O_DIM], f32, tag="onm")
            for j in range(ngt):
                pt = ppsum.tile([128, 128], bf16, tag="work")
                nc.tensor.transpose(pt[:, 0:O_DIM], outT[:, j * 128:(j + 1) * 128],
                                    ident_bf[0:O_DIM, 0:O_DIM])
                nc.scalar.copy(onm[:, j * O_DIM:(j + 1) * O_DIM], pt[:, 0:O_DIM])
            nfull = G // 128
            if nfull:
                nc.sync.dma_start(
                    out=out_d.ap()[0:nfull * 128, :].rearrange(
                        "(s p) o -> p s o", p=128),
                    in_=onm[:, :nfull * O_DIM].rearrange(
                        "p (s o) -> p s o", o=O_DIM))
            rem = G - nfull * 128
            if rem:
                nc.sync.dma_start(out=out_d.ap()[nfull * 128:G, :],
                                  in_=onm[0:rem, nfull * O_DIM:(nfull + 1) * O_DIM])

    nc.compile()
    return nc


def _build_in_maps(inputs, dims, arrays):
    import ml_dtypes
    f = lambda x: np.ascontiguousarray(np.asarray(x, np.float32))
    col = lambda x: f(x).reshape(-1, 1)
    shared = {
        "iota": 0,  # placeholder, set below (bf16)
        "iota16": np.tile(np.arange(WWIN, dtype=np.int16), (128, 1)),
        "W1a": f(inputs["W1a"]), "W1b": f(inputs["W1b"]),
        "W2a": f(inputs["W2a"]), "W2b": f(inputs["W2b"]),
        "Wf1": f(inputs["Wf1"]), "Wf2": f(inputs["Wf2"]),
        "Wc1": f(inputs["Wc1"]), "Wc2": f(inputs["Wc2"]),
        "b1a": col(inputs["b1a"]), "b1b": col(inputs["b1b"]),
        "b2a": col(inputs["b2a"]), "b2b": col(inputs["b2b"]),
        "bf1": col(inputs["bf1"]), "bf2": col(inputs["bf2"]),
        "bc1": col(inputs["bc1"]), "bc2": col(inputs["bc2"]),
        "gc": col(inputs["gc"]), "bec": col(inputs["bec"]),
        "rmc": col(inputs["rmc"]), "rvc": col(inputs["rvc"]),
        "a_prelu_v": np.full((HC_DIM, 1),
                             np.float32(np.asarray(inputs["a_prelu"]))),
    }
    for pfx in ["n1_", "n2_", "f1_", "f2_"]:
        for sfx in ["g", "b", "rm", "rv"]:
            shared[pfx + sfx] = col(inputs[pfx + sfx])
    shared["iota"] = np.tile(np.arange(128).astype(ml_dtypes.bfloat16),
                             (128, 1))
    in_maps = []
    for k in range(NCORES):
        m = dict(shared)
        m["posE"] = arrays["posE"][k].astype(ml_dtypes.bfloat16)
        m["pos_nm"] = arrays["pos_nm"][k].astype(ml_dtypes.bfloat16)
        m["dwc1"] = arrays["dwc1"][k].astype(ml_dtypes.bfloat16)
        m["gidx2"] = arrays["gidx2"][k]
        m["dwc2"] = arrays["dwc2"][k]
        m["Bpool"] = arrays["Bpool"][k]
        in_maps.append(m)
    return in_maps


def _get_compiled(pos, edge_index, batch, N, E, G):
    dims, arrays = _preprocess(pos, edge_index, batch, N, E, G)
    key = tuple(sorted((k, str(v)) for k, v in dims.items()))
    if key not in _CACHE:
        _CACHE[key] = _build_program(dims)
    return _CACHE[key], dims, arrays


def kernel(**inputs):
    from concourse.bass_utils import run_bass_kernel_spmd
    pos = np.asarray(inputs["pos"])
    ei = np.asarray(inputs["edge_index"])
    batch = np.asarray(inputs["batch"])
    nc, dims, arrays = _get_compiled(pos, ei, batch, pos.shape[0],
                                     ei.shape[1], G_FULL)
    in_maps = _build_in_maps(inputs, dims, arrays)
    res = run_bass_kernel_spmd(nc, in_maps, list(range(NCORES)))
    return np.asarray(res.results[0]["out"], np.float32)

